# revision 1
# baseline (speedup 1.0000x reference)
"""Trainium2 Bass kernel for the sliding-window bidirectional-LSTM "CNN".

Self-contained: hardcodes shapes/sharding for the nn_CNN problem
(S=256, B=32, F=16, H=128, E=128, OUT=5, V=50257, 8 cores).

Strategy (per core k of 8):
  - chunks n in [31k, 31k+30] (clamped to 240; clamped duplicates are
    excluded from the max-pool via masks), 992 columns = 31 chunks x 32 batch,
    H=128 on partitions.
  - embedding gather on device via indirect_copy from a host-dedup'd
    transposed table [E=128, <=1472 tokens].
  - XG = W_ih.x + b precomputed for both directions over the 46 positions
    the core needs; the per-step input is a 32-column shifted slice.
  - forward: run 16 steps, capture h at t == len-1 via copy_predicated.
  - backward: zero XG_b (incl. bias) at positions >= len  ->  state stays
    exactly 0 until the chunk "starts"; runs positions high->low; tail
    chunks (n >= len, single step) patched by one elementwise pass.
  - per step x dir: 4x matmul W_hh.h + 4x identity-matmul XG accumulation
    into PSUM [128, 4, 1024]; ACT sigmoid over (i,f,o) batch + tanh(g);
    DVE cell ops; ACT tanh(c); DVE h = o*tanh(c).
  - max-pool over chunks on device -> per-core partials [2, 128, 32];
    final 8-way max-combine + 5-dim FC on host (output [32, 5]).
"""

import numpy as np

import concourse.bass as bass
import concourse.tile as tile
import concourse.mybir as mybir
from concourse import bass2jax

# ---------------------------------------------------------------- constants
S, B, F, H, E, OUT, V = 256, 32, 16, 128, 128, 5, 50257
NCOREs = 8
NCH = 241            # chunks total
CPC = 31             # chunks per core
COLS = CPC * B       # 992
NPOS = CPC + F - 1   # 46 positions per core
PCOLS = NPOS * B     # 1472
GPERM = [0, 1, 3, 2]  # device gate order (i, f, o, g) <- reference (i, f, g, o)
NEG = -1.0e30

_FP32 = mybir.dt.float32
_F32R = mybir.dt.float32r
_BF16 = mybir.dt.bfloat16
_U8 = mybir.dt.uint8
_U16 = mybir.dt.uint16


# ---------------------------------------------------------------- walrus fix
# This walrus build supports exactly ONE sync-wait per instruction; Tile
# attaches several. Hoist extras onto same-engine NoOps placed just before.
_ws_counter = [0]


def _split_multi_waits(nc):
    for f in nc.m.functions:
        for bb in f.blocks:
            out = []
            for inst in bb.instructions:
                si = inst.sync_info
                if si is not None and si.on_wait and len(si.on_wait) > 1:
                    waits = list(si.on_wait)
                    for w in waits[:-1]:
                        _ws_counter[0] += 1
                        nop = mybir.InstNoOp(
                            name=f"I-waitsplit-{_ws_counter[0]}",
                            opcode="NoOp",
                            engine=inst.engine,
                            debug=inst.debug,
                            ins=[],
                            outs=[],
                        )
                        nop.sync_info = mybir.SyncInfo(on_wait=[w], on_update=[])
                        out.append(nop)
                    si.on_wait.clear()
                    si.on_wait.append(waits[-1])
                out.append(inst)
            bb.instructions[:] = out


# ---------------------------------------------------------------- program
def build_program(dt_mm=_BF16, dt_el=_FP32, reps=1):
    """Build the SPMD single-core Bass program. Returns nc."""
    f32 = _FP32
    dt = dt_el
    dtm = dt_mm
    r32 = (dt_mm == _F32R)

    def v32(ap):
        return ap.bitcast(f32) if r32 else ap
    nc = bass.Bass("TRN2", target_bir_lowering=False, debug=False,
                   num_devices=NCOREs)

    def din(name, shape, dtype):
        return nc.declare_dram_parameter(name, list(shape), dtype, isOutput=False)

    tab_in = din("tab", [128, PCOLS], f32)
    gidx_in = din("gidx", [128, PCOLS // 16], _U16)
    whhT_f_in = din("whhT_f", [128, 4 * H], dtm)
    whhT_b_in = din("whhT_b", [128, 4 * H], dtm)
    wihT_f_in = din("wihT_f", [128, 4 * H], dtm)
    wihT_b_in = din("wihT_b", [128, 4 * H], dtm)
    bias_f_in = din("bias_f", [128, 4], f32)
    bias_b_in = din("bias_b", [128, 4], f32)
    ident_in = din("ident", [128, 128], dtm)
    cmask_in = din("cmask", [128, F, COLS], _U8)
    zmask_in = din("zmask", [128, PCOLS], dt)
    tailmask_in = din("tailmask", [128, COLS], dt)
    dupmask_in = din("dupmask", [128, COLS], _U8)
    pool_out = nc.declare_dram_parameter("pool", [128, 2, B], f32, isOutput=True)

    with tile.TileContext(nc) as tc:
        import contextlib
        with contextlib.ExitStack() as ctx:
            const = ctx.enter_context(tc.tile_pool(name="const", bufs=1))
            big = ctx.enter_context(tc.tile_pool(name="big", bufs=1))
            state = ctx.enter_context(tc.tile_pool(name="state", bufs=1))
            work = ctx.enter_context(tc.tile_pool(name="work", bufs=2))
            ps = ctx.enter_context(tc.tile_pool(name="ps", bufs=2, space="PSUM"))

            # ---- constant loads (once) ----
            t_whhT = {}
            t_wihT = {}
            t_bias = {}
            for dirn, w_in, wi_in, b_in in (
                ("f", whhT_f_in, wihT_f_in, bias_f_in),
                ("b", whhT_b_in, wihT_b_in, bias_b_in),
            ):
                t_whhT[dirn] = const.tile([128, 4 * H], dtm, tag=f"whhT_{dirn}", name=f"whhT_{dirn}")
                nc.sync.dma_start(out=t_whhT[dirn][:], in_=w_in[:])
                t_wihT[dirn] = const.tile([128, 4 * H], dtm, tag=f"wihT_{dirn}", name=f"wihT_{dirn}")
                nc.sync.dma_start(out=t_wihT[dirn][:], in_=wi_in[:])
                t_bias[dirn] = const.tile([128, 4], f32, tag=f"bias_{dirn}", name=f"bias_{dirn}")
                nc.sync.dma_start(out=t_bias[dirn][:], in_=b_in[:])
            t_ident = const.tile([128, 128], dtm, tag="ident", name="ident")
            nc.sync.dma_start(out=t_ident[:], in_=ident_in[:])

            for rep in range(reps):
                # ---- per-iteration input loads ----
                t_tab = big.tile([128, PCOLS], f32, tag="tab", name="tab")
                nc.sync.dma_start(out=t_tab[:], in_=tab_in[:])
                t_gidx = big.tile([128, PCOLS // 16], _U16, tag="gidx", name="gidx")
                nc.sync.dma_start(out=t_gidx[:], in_=gidx_in[:])
                t_zmask = big.tile([128, PCOLS], dt, tag="zmask", name="zmask")
                nc.sync.dma_start(out=t_zmask[:], in_=zmask_in[:])
                t_tailmask = big.tile([128, COLS], dt, tag="tailmask", name="tailmask")
                nc.sync.dma_start(out=t_tailmask[:], in_=tailmask_in[:])
                t_dupmask = big.tile([128, COLS], _U8, tag="dupmask", name="dupmask")
                nc.sync.dma_start(out=t_dupmask[:], in_=dupmask_in[:])

                # ---- gather x.T  [E=128, PCOLS] ----
                t_xT32 = big.tile([128, PCOLS, 1], f32, tag="xT32", name="xT32")
                for s0 in range(0, PCOLS, 736):
                    s1 = min(s0 + 736, PCOLS)
                    nc.gpsimd.indirect_copy(
                        t_xT32[:, s0:s1, :], t_tab[:],
                        t_gidx[:, s0 // 16:(s1 + 15) // 16],
                        i_know_ap_gather_is_preferred=True)
                if dtm == f32:
                    t_xT = t_xT32[:, :, 0]
                else:
                    t_xTc = big.tile([128, PCOLS], dtm, tag="xTc", name="xTc")
                    nc.vector.tensor_copy(t_xTc[:], t_xT32[:, :, 0])
                    t_xT = t_xTc[:]

                # ---- XG precompute: [128, 4, PCOLS] per direction ----
                t_XG = {}
                for dirn in ("f", "b"):
                    t_XG[dirn] = big.tile([128, 4, PCOLS], dtm, tag=f"XG_{dirn}", name=f"XG_{dirn}")
                    for s0 in range(0, PCOLS, 512):
                        s1 = min(s0 + 512, PCOLS)
                        psx = ps.tile([128, 4, 512], f32, tag="ps", name="ps")
                        for g in range(4):
                            nc.tensor.matmul(
                                psx[:, g, 0:s1 - s0],
                                t_wihT[dirn][:, g * H:(g + 1) * H],
                                t_xT[:, s0:s1],
                                start=True, stop=True)
                        for g in range(4):
                            if g % 2 == 0:
                                nc.scalar.activation(
                                    t_XG[dirn][:, g, s0:s1],
                                    psx[:, g, 0:s1 - s0],
                                    mybir.ActivationFunctionType.Identity,
                                    bias=t_bias[dirn][:, g:g + 1])
                            else:
                                nc.vector.tensor_scalar_add(
                                    t_XG[dirn][:, g, s0:s1],
                                    psx[:, g, 0:s1 - s0],
                                    t_bias[dirn][:, g:g + 1])

                # ---- tail patch (before zeroing XG_b): single-step LSTM on
                #      raw XG_b at the chunk-start position (cols 0:COLS) ----
                Sig = mybir.ActivationFunctionType.Sigmoid
                Tanh = mybir.ActivationFunctionType.Tanh
                tl_i = work.tile([128, COLS], dt, tag="tm1", name="tl_i")
                nc.scalar.activation(tl_i[:], v32(t_XG["b"][:, 0, 0:COLS]), Sig)
                tl_g = work.tile([128, COLS], dt, tag="tm2", name="tl_g")
                nc.scalar.activation(tl_g[:], v32(t_XG["b"][:, 3, 0:COLS]), Tanh)
                tl_o = work.tile([128, COLS], dt, tag="gct", name="tl_o")
                nc.scalar.activation(tl_o[:], v32(t_XG["b"][:, 2, 0:COLS]), Sig)
                tl_c = work.tile([128, COLS], dt, tag="tct", name="tl_c")
                nc.vector.tensor_mul(tl_c[:], tl_i[:], tl_g[:])
                nc.scalar.activation(tl_c[:], tl_c[:], Tanh)
                t_htail = big.tile([128, COLS], dt, tag="htail", name="htail")
                nc.vector.tensor_mul(t_htail[:], tl_o[:], tl_c[:])
                nc.vector.tensor_mul(t_htail[:], t_htail[:], t_tailmask[:])

                # ---- zero XG_b at positions >= len (incl. bias) ----
                for g in range(4):
                    nc.vector.tensor_mul(
                        t_XG["b"][:, g, :], v32(t_XG["b"][:, g, :]), t_zmask[:])

                # ---- state init ----
                hdt = dtm if r32 else dt
                Cpy = mybir.ActivationFunctionType.Copy

                def setconst(ap, val):
                    # f32r-typed tiles need a "rounding" writer for walrus
                    if ap.dtype == _F32R:
                        nc.scalar.activation(ap, t_zmask[:, 0:ap.shape[-1]],
                                             Cpy, scale=0.0, bias=float(val))
                    else:
                        nc.vector.memset(ap, float(val))

                t_h = {}
                t_c = {}
                t_hmm = {}
                for dirn in ("f", "b"):
                    t_h[dirn] = state.tile([128, COLS], hdt, tag=f"h_{dirn}", name=f"h_{dirn}")
                    setconst(t_h[dirn][:], 0.0)
                    t_c[dirn] = state.tile([128, COLS], dt, tag=f"c_{dirn}", name=f"c_{dirn}")
                    nc.vector.memset(t_c[dirn][:], 0.0)
                    if dtm != dt and not r32:
                        t_hmm[dirn] = state.tile([128, COLS], dtm, tag=f"hmm_{dirn}", name=f"hmm_{dirn}")
                        nc.vector.memset(t_hmm[dirn][:], 0.0)
                    else:
                        t_hmm[dirn] = t_h[dirn]
                t_hacc = state.tile([128, COLS], dt, tag="hacc", name="hacc")
                nc.vector.memset(t_hacc[:], NEG)

                # ---- main loop: 16 steps x 2 directions ----
                for t in range(F):
                    t_cmt = work.tile([128, COLS], _U8, tag="cmt", name="cmt")
                    nc.sync.dma_start(out=t_cmt[:], in_=cmask_in[:, t, :])
                    for dirn in ("f", "b"):
                        off = (t if dirn == "f" else (F - 1 - t)) * B
                        h, c = t_h[dirn], t_c[dirn]
                        ifo = work.tile([128, 3, COLS], dt, tag="ifo", name="ifo")
                        gct = work.tile([128, COLS], dt, tag="gct", name="gct")
                        for s0, s1 in ((0, 512), (512, COLS)):
                            psg = ps.tile([128, 4, 512], f32, tag="ps", name="ps")
                            for g in range(4):
                                nc.tensor.matmul(
                                    psg[:, g, 0:s1 - s0],
                                    t_whhT[dirn][:, g * H:(g + 1) * H],
                                    t_hmm[dirn][:, s0:s1],
                                    start=True, stop=False)
                            for g in range(4):
                                nc.tensor.matmul(
                                    psg[:, g, 0:s1 - s0],
                                    t_ident[:],
                                    t_XG[dirn][:, g, off + s0:off + s1],
                                    start=False, stop=True)
                            nc.scalar.activation(
                                ifo[:, :, s0:s1], psg[:, 0:3, 0:s1 - s0], Sig)
                            nc.scalar.activation(
                                gct[:, s0:s1], psg[:, 3, 0:s1 - s0], Tanh)
                        tm1 = work.tile([128, COLS], dt, tag="tm1", name="tm1")
                        nc.vector.tensor_mul(tm1[:], ifo[:, 1, :], c[:])
                        tm2 = work.tile([128, COLS], dt, tag="tm2", name="tm2")
                        nc.gpsimd.tensor_mul(tm2[:], ifo[:, 0, :], gct[:])
                        nc.vector.tensor_add(c[:], tm1[:], tm2[:])
                        tct = work.tile([128, COLS], dt, tag="tct", name="tct")
                        nc.scalar.activation(tct[:], c[:], Tanh)
                        nc.vector.tensor_mul(h[:], ifo[:, 2, :], tct[:])
                        if dtm != dt and not r32:
                            nc.vector.tensor_copy(t_hmm[dirn][:], h[:])
                        if dirn == "f":
                            nc.vector.copy_predicated(
                                t_hacc[:], t_cmt[:], v32(h[:]))

                # ---- epilogue ----
                t_hbp = work.tile([128, COLS], dt, tag="tm2", name="hbp")
                nc.vector.tensor_add(t_hbp[:], v32(t_h["b"][:]), t_htail[:])
                t_negc = work.tile([128, COLS], dt, tag="tm1", name="negc")
                nc.vector.memset(t_negc[:], NEG)
                nc.vector.copy_predicated(t_hbp[:], t_dupmask[:], t_negc[:])

                t_pool = work.tile([128, 2, B], f32, tag="pool", name="pool")
                nc.vector.tensor_reduce(
                    t_pool[:, 0, :],
                    t_hacc[:].rearrange("p (n b) -> p b n", b=B),
                    axis=mybir.AxisListType.X, op=mybir.AluOpType.max)
                nc.vector.tensor_reduce(
                    t_pool[:, 1, :],
                    t_hbp[:].rearrange("p (n b) -> p b n", b=B),
                    axis=mybir.AxisListType.X, op=mybir.AluOpType.max)
                nc.sync.dma_start(out=pool_out[:], in_=t_pool[:])

    return nc


# ---------------------------------------------------------------- host prep
def host_inputs(text, text_lengths, emb, w_ih_f, w_hh_f, b_f,
                w_ih_b, w_hh_b, b_b, dtm_np, dte_np):
    """Build the 8 per-core input dicts."""
    text = np.asarray(text).astype(np.int64)            # [S, B]
    L = np.asarray(text_lengths).astype(np.int64)       # [B]
    emb = np.asarray(emb, dtype=np.float32)

    def wT(w):  # [4H, X] -> [X, 4H] with device gate order (i, f, o, g)
        t = np.ascontiguousarray(w.astype(np.float32).T)
        return np.concatenate([t[:, g * H:(g + 1) * H] for g in GPERM], axis=1)

    def bcols(b):
        b = np.asarray(b, dtype=np.float32)
        return np.stack([b[g * H:(g + 1) * H] for g in GPERM], axis=1)  # [128,4]

    def wrap_idx(idx):
        n = len(idx)
        cols = (n + 15) // 16
        pad = np.zeros(cols * 16, dtype=np.uint16)
        pad[:n] = idx
        return np.tile(pad.reshape(cols, 16).T, (8, 1))  # [128, cols]

    common = dict(
        whhT_f=wT(w_hh_f).astype(dtm_np), whhT_b=wT(w_hh_b).astype(dtm_np),
        wihT_f=wT(w_ih_f).astype(dtm_np), wihT_b=wT(w_ih_b).astype(dtm_np),
        bias_f=bcols(b_f), bias_b=bcols(b_b),
        ident=np.eye(128, dtype=np.float32).astype(dtm_np),
    )

    in_maps = []
    for k in range(NCOREs):
        n0 = CPC * k
        j = np.arange(CPC)
        n_eff = np.minimum(n0 + j, NCH - 1)             # [31]
        p_idx = np.arange(NPOS)
        pos_eff = np.minimum(n0 + p_idx, S - 1)         # [46]

        toks = text[pos_eff, :]                          # [46, B]
        uniq, ranks = np.unique(toks.ravel(), return_inverse=True)
        tab = np.zeros((128, PCOLS), dtype=np.float32)
        tab[:, :len(uniq)] = emb[uniq].T
        gidx = wrap_idx(ranks.astype(np.uint16))

        l = np.clip(L[None, :] - n_eff[:, None], 1, F)   # [31, B]
        dup = (n0 + j > NCH - 1)[:, None] & np.ones((1, B), bool)
        cmask = np.zeros((F, CPC, B), dtype=np.uint8)
        for t in range(F):
            cmask[t] = ((l == t + 1) & ~dup).astype(np.uint8)
        zmask = (pos_eff[:, None] < L[None, :]).astype(dtm_np)      # [46, B]
        tailmask = (n_eff[:, None] >= L[None, :]).astype(dte_np)    # [31, B]

        m = dict(common)
        m["tab"] = tab
        m["gidx"] = gidx
        m["cmask"] = np.broadcast_to(
            cmask.reshape(1, F, COLS), (128, F, COLS)).copy()
        m["zmask"] = np.broadcast_to(
            zmask.reshape(1, PCOLS), (128, PCOLS)).copy()
        m["tailmask"] = np.broadcast_to(
            tailmask.reshape(1, COLS), (128, COLS)).copy()
        m["dupmask"] = np.broadcast_to(
            dup.astype(np.uint8).reshape(1, COLS), (128, COLS)).copy()
        in_maps.append(m)
    return in_maps


def host_finish(pools, w_fc, b_fc):
    """pools: list of 8 arrays [128, 2, B] -> output [B, OUT] fp32."""
    allp = np.stack(pools, axis=0)                       # [8, 128, 2, B]
    red = allp.max(axis=0)                               # [128, 2, B]
    hid = np.concatenate([red[:, 0, :].T, red[:, 1, :].T], axis=1)  # [B, 2H]
    w_fc = np.asarray(w_fc, dtype=np.float32)
    b_fc = np.asarray(b_fc, dtype=np.float32)
    return (hid @ w_fc.T + b_fc).astype(np.float32)


# ---------------------------------------------------------------- runner
_CACHE = {}


def get_runner(dt_mm=_BF16, dt_el=_FP32, reps=1):
    key = (str(dt_mm), str(dt_el), reps)
    if key not in _CACHE:
        nc = build_program(dt_mm=dt_mm, dt_el=dt_el, reps=reps)
        _split_multi_waits(nc)
        _CACHE[key] = nc
    return _CACHE[key]


def run_on_device(nc, in_maps):
    res = bass2jax.run_bass_via_pjrt(nc, in_maps, n_cores=NCOREs)
    return [r["pool"] for r in res]


def kernel(text, text_lengths, emb, w_ih_f, w_hh_f, b_f,
           w_ih_b, w_hh_b, b_b, w_fc, b_fc):
    nc = get_runner(dt_mm=_F32R, dt_el=_FP32, reps=1)
    in_maps = host_inputs(text, text_lengths, emb, w_ih_f, w_hh_f, b_f,
                          w_ih_b, w_hh_b, b_b, np.float32, np.float32)
    pools = run_on_device(nc, in_maps)
    return host_finish(pools, w_fc, b_fc)



# revision 32
# speedup vs baseline: 2.4721x; 2.4721x over previous
"""Trainium2 Bass kernel for the sliding-window bidirectional-LSTM "CNN".

Self-contained: hardcodes shapes for the nn_CNN problem
(S=256, B=32, F=16, H=128, E=128, OUT=5, V=50257, 8 cores).

Run-packed strategy (v2): only chunks n < L_b are computed on device.
  - a "run" = R=8 consecutive chunks of ONE batch b -> 8 columns.
  - runs tile [0, ceil(min(L_b,241)/8)*8) per batch; all runs distributed
    over 8 cores; padded to 64 runs/core -> C=512 columns per core
    (PSUM: each gate x 512 cols = exactly one 2KB bank).
  - XG = W_ih.x + b precomputed per run over R+15=23 positions
    ([128, 4, r_pc, 23] bf16); the step-t matmul rhs is the 2-level AP
    [:, g, :, t:t+8] (forward) or [:, g, :, 15-t:23-t] (backward).
  - bias folded at XG production; backward XG additionally multiplied by
    zmask (positions >= L_b -> 0) via one fused scalar_tensor_tensor.
  - forward: capture h at t == l-1 via per-step copy_predicated masks
    (full cols capture at t=15, edge cols earlier). bf16 elementwise.
  - backward: runs 16 steps positions 15..0; zmask keeps state 0 until
    the valid suffix begins; final h is the answer; tail/pad columns
    forced to -1e30 before pooling.
  - per-run max-pool on device -> [128, 2, r_pc] partials; host maps
    runs->batches, adds the tail-chunk contribution (chunks n >= L_b are
    single-step cells of position n only -- batch-independent, done on
    host as a suffix-max over positions), and applies the 5-dim FC.
"""

import numpy as np

import concourse.bass as bass
import concourse.tile as tile
import concourse.mybir as mybir
from concourse import bass2jax

# ---------------------------------------------------------------- constants
S, B, F, H, E, OUT, V = 256, 32, 16, 128, 128, 5, 50257
NCOREs = 8
NCH = S - F + 1      # 241 chunks total
R = 8                # chunks per run
PB = R + F - 1       # 23 positions per run block
GPERM = [0, 1, 3, 2]  # device gate order (i, f, o, g) <- reference (i, f, g, o)
NEG = -1.0e30
CW = 512             # columns per matmul chunk (one PSUM bank per gate)
RPC_CHUNK = CW // R  # 64 runs per 512-col chunk

_FP32 = mybir.dt.float32
_F32R = mybir.dt.float32r
_BF16 = mybir.dt.bfloat16
_U16 = mybir.dt.uint16
_U8 = mybir.dt.uint8

Sig = mybir.ActivationFunctionType.Sigmoid
Tanh = mybir.ActivationFunctionType.Tanh
Ident = mybir.ActivationFunctionType.Identity


# ---------------------------------------------------------------- walrus fix
# This walrus build supports exactly ONE sync-wait per instruction; Tile
# attaches several. Hoist extras onto same-engine NoOps placed just before.
_ws_counter = [0]


def _split_multi_waits(nc):
    for f in nc.m.functions:
        for bb in f.blocks:
            out = []
            for inst in bb.instructions:
                si = inst.sync_info
                if si is not None and si.on_wait and len(si.on_wait) > 1:
                    waits = list(si.on_wait)
                    for w in waits[:-1]:
                        _ws_counter[0] += 1
                        nop = mybir.InstNoOp(
                            name=f"I-waitsplit-{_ws_counter[0]}",
                            opcode="NoOp",
                            engine=inst.engine,
                            debug=inst.debug,
                            ins=[],
                            outs=[],
                        )
                        nop.sync_info = mybir.SyncInfo(on_wait=[w], on_update=[])
                        out.append(nop)
                    si.on_wait.clear()
                    si.on_wait.append(waits[-1])
                out.append(inst)
            bb.instructions[:] = out


# ---------------------------------------------------------------- planning
def make_plan(text_lengths):
    """Distribute runs across cores. Returns (runs_per_core, r_pc)."""
    L = np.asarray(text_lengths).astype(np.int64)
    runs = []
    for b in range(B):
        lb = int(min(L[b], NCH))
        for n0 in range(0, ((lb + R - 1) // R) * R, R):
            runs.append((b, n0))
    r_pc = -(-len(runs) // NCOREs)              # ceil
    r_pc = -(-r_pc // RPC_CHUNK) * RPC_CHUNK    # pad to multiple of 64
    per_core = [runs[k::NCOREs] for k in range(NCOREs)]  # round-robin
    return per_core, r_pc


# ---------------------------------------------------------------- program
def build_program(r_pc, reps=1):
    """SPMD single-core program for r_pc runs (C = 8*r_pc cols)."""
    f32 = _FP32
    bf = _BF16
    C = R * r_pc
    XCOLS = PB * r_pc                    # XG flat columns
    n512 = C // CW                       # 512-col chunks
    assert r_pc % RPC_CHUNK == 0

    nc = bass.Bass("TRN2", target_bir_lowering=False, debug=False,
                   num_devices=NCOREs)

    def din(name, shape, dtype):
        return nc.declare_dram_parameter(name, list(shape), dtype, isOutput=False)

    tab_in = din("tab", [128, XCOLS], bf)
    gidx_in = din("gidx", [128, XCOLS // 16], _U16)
    whhT_f_in = din("whhT_f", [128, 4 * H], bf)
    whhT_b_in = din("whhT_b", [128, 4 * H], bf)
    wihT_f_in = din("wihT_f", [128, 4 * H], bf)
    wihT_b_in = din("wihT_b", [128, 4 * H], bf)
    bias_f_in = din("bias_f", [128, 4], f32)
    bias_b_in = din("bias_b", [128, 4], f32)
    ident_in = din("ident", [128, 128], bf)
    cmask_in = din("cmask", [128, F, C], _U8)
    zmask_in = din("zmask", [128, XCOLS], f32)
    tailneg_in = din("tailneg", [128, C], _U8)
    pool_out = nc.declare_dram_parameter("pool", [128, 2, r_pc], f32,
                                         isOutput=True)

    def r32(ap):
        return ap.bitcast(_F32R)

    with tile.TileContext(nc) as tc:
        import contextlib
        with contextlib.ExitStack() as ctx:
            const = ctx.enter_context(tc.tile_pool(name="const", bufs=1))
            big = ctx.enter_context(tc.tile_pool(name="big", bufs=2))
            state = ctx.enter_context(tc.tile_pool(name="state", bufs=2))
            work = ctx.enter_context(tc.tile_pool(name="work", bufs=2))
            ps = ctx.enter_context(tc.tile_pool(name="ps", bufs=2, space="PSUM"))

            # ---- constant loads (once) ----
            t_whhT = {}
            t_wihT = {}
            t_bias = {}
            for dirn, w_in, wi_in, b_in in (
                ("f", whhT_f_in, wihT_f_in, bias_f_in),
                ("b", whhT_b_in, wihT_b_in, bias_b_in),
            ):
                t_whhT[dirn] = const.tile([128, 4 * H], bf, tag=f"whhT_{dirn}", name=f"whhT_{dirn}")
                nc.sync.dma_start(out=t_whhT[dirn][:], in_=w_in[:])
                t_wihT[dirn] = const.tile([128, 4 * H], bf, tag=f"wihT_{dirn}", name=f"wihT_{dirn}")
                nc.sync.dma_start(out=t_wihT[dirn][:], in_=wi_in[:])
                t_bias[dirn] = const.tile([128, 4], f32, tag=f"bias_{dirn}", name=f"bias_{dirn}")
                nc.sync.dma_start(out=t_bias[dirn][:], in_=b_in[:])
            t_ident = const.tile([128, 128], bf, tag="ident", name="ident")
            nc.sync.dma_start(out=t_ident[:], in_=ident_in[:])

            # PE warmup: ramp the tensor engine to full clock while the
            # input DMAs land (outputs are never read).
            t_warm = const.tile([128, CW], bf, tag="warm", name="warm")
            nc.vector.memset(t_warm[:], 0.0)
            psw = ps.tile([128, 4, CW], f32, tag="ps", name="psw")
            for w in range(12):
                nc.tensor.matmul(psw[:, w % 4, :], t_whhT["f"][:, 0:H],
                                 t_warm[:], start=True, stop=True)

            def make_prologue(rep):
                """Per-rep input load + XG production, split into parts
                that can be interleaved with the previous rep's steps."""
                hd = {}
                parts = []

                def p_dma():
                    hd["tab"] = big.tile([128, XCOLS], bf, tag="tab", name="tab")
                    nc.sync.dma_start(out=hd["tab"][:], in_=tab_in[:])
                    hd["gidx"] = big.tile([128, XCOLS // 16], _U16, tag="gidx", name="gidx")
                    nc.sync.dma_start(out=hd["gidx"][:], in_=gidx_in[:])
                    hd["zmask"] = big.tile([128, XCOLS], f32, tag="zmask", name="zmask")
                    nc.sync.dma_start(out=hd["zmask"][:], in_=zmask_in[:])
                    hd["cmask"] = big.tile([128, F, C], _U8, tag="cmask", name="cmask")
                    nc.sync.dma_start(out=hd["cmask"][:], in_=cmask_in[:])
                    hd["tailneg"] = big.tile([128, C], _U8, tag="tailneg", name="tailneg")
                    nc.sync.dma_start(out=hd["tailneg"][:], in_=tailneg_in[:])
                    hd["xT32"] = big.tile([128, XCOLS, 1], bf, tag="xT32", name="xT32")
                    hd["XG_f"] = big.tile([128, 4, XCOLS], bf, tag="XG_f", name="XG_f")
                    hd["XG_b"] = big.tile([128, 4, XCOLS], bf, tag="XG_b", name="XG_b")
                parts.append(p_dma)

                for s0g in range(0, XCOLS, CW):
                    def p_gather(s0=s0g):
                        s1 = min(s0 + CW, XCOLS)
                        nc.gpsimd.indirect_copy(
                            hd["xT32"][:, s0:s1, :], hd["tab"][:],
                            hd["gidx"][:, s0 // 16:(s1 + 15) // 16],
                            i_know_ap_gather_is_preferred=True)
                    parts.append(p_gather)

                for dirn in ("f", "b"):
                    for s0x in range(0, XCOLS, CW):
                        def p_xg(dirn=dirn, s0=s0x):
                            s1 = min(s0 + CW, XCOLS)
                            xT = hd["xT32"][:, :, 0]
                            psx = ps.tile([128, 4, CW], f32, tag="ps", name="ps")
                            for g in range(4):
                                nc.tensor.matmul(
                                    psx[:, g, 0:s1 - s0],
                                    t_wihT[dirn][:, g * H:(g + 1) * H],
                                    xT[:, s0:s1],
                                    start=True, stop=True)
                            for g in range(4):
                                if dirn == "f":
                                    nc.vector.tensor_scalar_add(
                                        hd["XG_f"][:, g, s0:s1],
                                        psx[:, g, 0:s1 - s0],
                                        t_bias[dirn][:, g:g + 1])
                                else:
                                    nc.vector.scalar_tensor_tensor(
                                        hd["XG_b"][:, g, s0:s1],
                                        psx[:, g, 0:s1 - s0],
                                        t_bias[dirn][:, g:g + 1],
                                        hd["zmask"][:, s0:s1],
                                        op0=mybir.AluOpType.add,
                                        op1=mybir.AluOpType.mult)
                        parts.append(p_xg)

                def p_state():
                    for dirn in ("f", "b"):
                        hd[f"h_{dirn}"] = state.tile([128, C], bf, tag=f"h_{dirn}", name=f"h_{dirn}")
                        nc.vector.memset(hd[f"h_{dirn}"][:], 0.0)
                        hd[f"c_{dirn}"] = state.tile([128, C], bf, tag=f"c_{dirn}", name=f"c_{dirn}")
                        nc.vector.memset(hd[f"c_{dirn}"][:], 0.0)
                    hd["hcap"] = state.tile([128, C], bf, tag="hcap", name="hcap")
                    nc.vector.memset(hd["hcap"][:], NEG)
                parts.append(p_state)
                return parts, hd

            def emit_rep(hd, inject):
                """One rep's 16x2 steps + epilogue; inject[t] = closures to
                emit after step t (next rep's prologue parts)."""
                xg_v = {d: hd[f"XG_{d}"][:].rearrange("p g (r s) -> p g r s", s=PB)
                        for d in ("f", "b")}
                for t in range(F):
                    for dirn in ("f", "b"):
                        off = t if dirn == "f" else (F - 1 - t)
                        h, c = hd[f"h_{dirn}"], hd[f"c_{dirn}"]
                        ifo = work.tile([128, 3, C], bf, tag="ifo", name="ifo")
                        gct = work.tile([128, C], bf, tag="gct", name="gct")
                        for cc in range(n512):
                            s0 = cc * CW
                            s1 = s0 + CW
                            r0 = cc * RPC_CHUNK
                            r1 = r0 + RPC_CHUNK
                            psg = ps.tile([128, 4, CW], f32, tag="ps", name="ps")
                            for g in (0, 1, 2, 3):
                                nc.tensor.matmul(
                                    psg[:, g, :],
                                    t_whhT[dirn][:, g * H:(g + 1) * H],
                                    h[:, s0:s1],
                                    start=True, stop=False)
                                nc.tensor.matmul(
                                    psg[:, g, :],
                                    t_ident[:],
                                    xg_v[dirn][:, g, r0:r1, off:off + R],
                                    start=False, stop=True)
                            # gates: sigmoid(i,f,o) after 6 mms, tanh(g)
                            # after 8
                            nc.scalar.activation(
                                ifo[:, :, s0:s1], psg[:, 0:3, :], Sig)
                            nc.scalar.activation(
                                gct[:, s0:s1], psg[:, 3, :], Tanh)
                        # c = f*c + i*g
                        tm1 = work.tile([128, C], bf, tag="tm1", name="tm1")
                        nc.vector.tensor_mul(tm1[:], ifo[:, 1, :], c[:])
                        tm2 = work.tile([128, C], bf, tag="tm2", name="tm2")
                        nc.gpsimd.tensor_mul(tm2[:], ifo[:, 0, :], gct[:])
                        nc.vector.tensor_add(c[:], tm1[:], tm2[:])
                        tct = work.tile([128, C], bf, tag="tct", name="tct")
                        nc.scalar.activation(tct[:], c[:], Tanh)
                        # h = o * tanh(c)
                        nc.vector.tensor_mul(h[:], ifo[:, 2, :], tct[:])
                        if dirn == "f":
                            nc.vector.copy_predicated(
                                hd["hcap"][:], hd["cmask"][:, t, :], h[:])
                    for fn in inject.get(t, []):
                        fn()

                # ---- epilogue ----
                t_negc = work.tile([128, C], bf, tag="tm1", name="negc")
                nc.vector.memset(t_negc[:], NEG)
                nc.vector.copy_predicated(hd["h_b"][:], hd["tailneg"][:], t_negc[:])

                t_pool = work.tile([128, 2, r_pc], f32, tag="pool", name="pool")
                nc.vector.tensor_reduce(
                    t_pool[:, 0, :],
                    hd["hcap"][:].rearrange("p (r e) -> p r e", e=R),
                    axis=mybir.AxisListType.X, op=mybir.AluOpType.max)
                nc.vector.tensor_reduce(
                    t_pool[:, 1, :],
                    hd["h_b"][:].rearrange("p (r e) -> p r e", e=R),
                    axis=mybir.AxisListType.X, op=mybir.AluOpType.max)
                nc.sync.dma_start(out=pool_out[:], in_=t_pool[:])

            parts, hd = make_prologue(0)
            for p in parts:
                p()
            for rep in range(reps):
                if rep + 1 < reps:
                    nparts, nhd = make_prologue(rep + 1)
                    # spread next-rep prologue over steps 4..4+len-1
                    inject = {4 + i: [p] for i, p in enumerate(nparts)}
                else:
                    nhd = None
                    inject = {}
                emit_rep(hd, inject)
                hd = nhd

    return nc


# ---------------------------------------------------------------- host prep
def _bf16(x):
    return np.asarray(x, dtype=mybir.dt.np(_BF16))


def wT(w, dt=np.float32):
    """[4H, X] -> [X, 4H] device gate order (i, f, g, o)."""
    t = np.ascontiguousarray(np.asarray(w, dtype=np.float32).T)
    return np.concatenate(
        [t[:, g * H:(g + 1) * H] for g in GPERM], axis=1).astype(dt)


def bcols(b):
    b = np.asarray(b, dtype=np.float32)
    return np.stack([b[g * H:(g + 1) * H] for g in GPERM], axis=1)  # [128,4]


def wrap_idx(idx, xcols):
    n = len(idx)
    cols = xcols // 16
    pad = np.zeros(cols * 16, dtype=np.uint16)
    pad[:n] = idx
    return np.tile(pad.reshape(cols, 16).T, (8, 1))  # [128, cols]


def host_inputs(text, text_lengths, emb, w_ih_f, w_hh_f, b_f,
                w_ih_b, w_hh_b, b_b):
    """Build per-core input dicts + the plan."""
    text = np.asarray(text).astype(np.int64)            # [S, B]
    L = np.asarray(text_lengths).astype(np.int64)       # [B]
    emb = np.asarray(emb, dtype=np.float32)

    per_core_runs, r_pc = make_plan(L)
    C = R * r_pc
    XCOLS = PB * r_pc

    bfnp = mybir.dt.np(_BF16)
    common = dict(
        whhT_f=wT(w_hh_f, bfnp), whhT_b=wT(w_hh_b, bfnp),
        wihT_f=wT(w_ih_f, bfnp), wihT_b=wT(w_ih_b, bfnp),
        bias_f=bcols(b_f), bias_b=bcols(b_b),
        ident=np.eye(128, dtype=np.float32).astype(bfnp),
    )

    in_maps = []
    for k in range(NCOREs):
        runs = per_core_runs[k]
        # slot positions [r_pc, PB] and their batch
        pos = np.zeros((r_pc, PB), dtype=np.int64)
        bat = np.zeros((r_pc,), dtype=np.int64)
        valid = np.zeros((r_pc,), dtype=bool)
        for j, (b, n0) in enumerate(runs):
            pos[j] = np.minimum(n0 + np.arange(PB), S - 1)
            bat[j] = b
            valid[j] = True
        toks = text[pos.ravel(), np.repeat(np.where(valid, bat, 0), PB)]
        uniq, ranks = np.unique(toks, return_inverse=True)
        assert len(uniq) <= XCOLS
        tab = np.zeros((128, XCOLS), dtype=np.float32)
        tab[:, :len(uniq)] = emb[uniq].T
        gidx = wrap_idx(ranks.astype(np.uint16), XCOLS)

        # zmask: position < L_b (per slot)
        zm = (pos < L[np.where(valid, bat, 0)][:, None]) & valid[:, None]
        zmask = zm.astype(np.float32).reshape(1, XCOLS)

        # per-column chunk index and capture length
        ncol = np.where(valid[:, None], pos[:, :R], NCH)  # [r_pc, R] chunk n
        Lb = L[np.where(valid, bat, 0)][:, None]
        l_eff = np.clip(Lb - ncol, None, F)               # <=0 -> tail
        is_tail = (ncol >= np.minimum(Lb, NCH)) | ~valid[:, None]
        cmask = np.zeros((F, r_pc, R), dtype=np.float32)
        for t in range(F):
            cmask[t] = ((l_eff == t + 1) & ~is_tail).astype(np.float32)
        tailneg = is_tail.astype(np.float32).reshape(1, C)

        m = dict(common)
        m["tab"] = _bf16(tab)
        m["gidx"] = gidx
        m["zmask"] = np.broadcast_to(zmask, (128, XCOLS)).copy()
        m["cmask"] = np.broadcast_to(
            cmask.reshape(1, F, C), (128, F, C)).astype(np.uint8)
        m["tailneg"] = np.broadcast_to(tailneg, (128, C)).astype(np.uint8)
        in_maps.append(m)
    return in_maps, per_core_runs, r_pc


# ---------------------------------------------------------------- host finish
def _cell1(xg):
    """Single-step LSTM cell from zero state; xg: [..., 4H] ref gate order."""
    i = 1.0 / (1.0 + np.exp(-xg[..., :H]))
    g = np.tanh(xg[..., 2 * H:3 * H])
    o = 1.0 / (1.0 + np.exp(-xg[..., 3 * H:]))
    c = i * g
    return o * np.tanh(c)


def host_finish(pools, per_core_runs, text, text_lengths, emb,
                w_ih_f, b_f, w_ih_b, b_b, w_fc, b_fc):
    text = np.asarray(text).astype(np.int64)
    L = np.asarray(text_lengths).astype(np.int64)
    emb = np.asarray(emb, dtype=np.float32)

    hf = np.full((128, B), NEG, dtype=np.float32)
    hb = np.full((128, B), NEG, dtype=np.float32)
    for k in range(NCOREs):
        p = np.asarray(pools[k], dtype=np.float32)      # [128, 2, r_pc]
        for j, (b, n0) in enumerate(per_core_runs[k]):
            np.maximum(hf[:, b], p[:, 0, j], out=hf[:, b])
            np.maximum(hb[:, b], p[:, 1, j], out=hb[:, b])

    # tail chunks: n in [L_b, 241) -> single-step cell of position n.
    Lmin = int(L.min())
    if Lmin < NCH:
        for b in range(B):
            lb = int(L[b])
            if lb >= NCH:
                continue
            nsb = np.arange(lb, NCH)
            x = emb[text[nsb, b]]                        # [nt, E]
            xg_f = x @ np.asarray(w_ih_f, np.float32).T + np.asarray(b_f, np.float32)
            xg_b = x @ np.asarray(w_ih_b, np.float32).T + np.asarray(b_b, np.float32)
            vf = _cell1(xg_f).max(axis=0)                # [H]
            vb = _cell1(xg_b).max(axis=0)
            np.maximum(hf[:, b], vf, out=hf[:, b])
            np.maximum(hb[:, b], vb, out=hb[:, b])

    hid = np.concatenate([hf.T, hb.T], axis=1)           # [B, 2H]
    w_fc = np.asarray(w_fc, dtype=np.float32)
    b_fc = np.asarray(b_fc, dtype=np.float32)
    return (hid @ w_fc.T + b_fc).astype(np.float32)


# ---------------------------------------------------------------- runner
_CACHE = {}


def get_runner(r_pc, reps=1):
    key = (r_pc, reps)
    if key not in _CACHE:
        nc = build_program(r_pc, reps=reps)
        _split_multi_waits(nc)
        _CACHE[key] = nc
    return _CACHE[key]


def run_on_device(nc, in_maps):
    res = bass2jax.run_bass_via_pjrt(nc, in_maps, n_cores=NCOREs)
    return [r["pool"] for r in res]


def kernel(text, text_lengths, emb, w_ih_f, w_hh_f, b_f,
           w_ih_b, w_hh_b, b_b, w_fc, b_fc):
    in_maps, per_core_runs, r_pc = host_inputs(
        text, text_lengths, emb, w_ih_f, w_hh_f, b_f, w_ih_b, w_hh_b, b_b)
    nc = get_runner(r_pc, reps=1)
    pools = run_on_device(nc, in_maps)
    return host_finish(pools, per_core_runs, text, text_lengths, emb,
                       w_ih_f, b_f, w_ih_b, b_b, w_fc, b_fc)


# revision 33
# speedup vs baseline: 2.6828x; 1.0852x over previous
"""Trainium2 Bass kernel for the sliding-window bidirectional-LSTM "CNN".

Self-contained: hardcodes shapes for the nn_CNN problem
(S=256, B=32, F=16, H=128, E=128, OUT=5, V=50257, 8 cores).

Run-packed strategy (v2): only chunks n < L_b are computed on device.
  - a "run" = R=8 consecutive chunks of ONE batch b -> 8 columns.
  - runs tile [0, ceil(min(L_b,241)/8)*8) per batch; all runs distributed
    over 8 cores; padded to 64 runs/core -> C=512 columns per core
    (PSUM: each gate x 512 cols = exactly one 2KB bank).
  - XG = W_ih.x + b precomputed per run over R+15=23 positions
    ([128, 4, r_pc, 23] bf16); the step-t matmul rhs is the 2-level AP
    [:, g, :, t:t+8] (forward) or [:, g, :, 15-t:23-t] (backward).
  - bias folded at XG production; backward XG additionally multiplied by
    zmask (positions >= L_b -> 0) via one fused scalar_tensor_tensor.
  - forward: capture h at t == l-1 via per-step copy_predicated masks
    (full cols capture at t=15, edge cols earlier). bf16 elementwise.
  - backward: runs 16 steps positions 15..0; zmask keeps state 0 until
    the valid suffix begins; final h is the answer; tail/pad columns
    forced to -1e30 before pooling.
  - per-run max-pool on device -> [128, 2, r_pc] partials; host maps
    runs->batches, adds the tail-chunk contribution (chunks n >= L_b are
    single-step cells of position n only -- batch-independent, done on
    host as a suffix-max over positions), and applies the 5-dim FC.
"""

import numpy as np

import concourse.bass as bass
import concourse.tile as tile
import concourse.mybir as mybir
from concourse import bass2jax

# ---------------------------------------------------------------- constants
S, B, F, H, E, OUT, V = 256, 32, 16, 128, 128, 5, 50257
NCOREs = 8
NCH = S - F + 1      # 241 chunks total
R = 8                # chunks per run
PB = R + F - 1       # 23 positions per run block
GPERM = [0, 1, 3, 2]  # device gate order (i, f, o, g) <- reference (i, f, g, o)
NEG = -1.0e30
CW = 512             # columns per matmul chunk (one PSUM bank per gate)
RPC_CHUNK = CW // R  # 64 runs per 512-col chunk

_FP32 = mybir.dt.float32
_F32R = mybir.dt.float32r
_BF16 = mybir.dt.bfloat16
_U16 = mybir.dt.uint16
_U8 = mybir.dt.uint8

Sig = mybir.ActivationFunctionType.Sigmoid
Tanh = mybir.ActivationFunctionType.Tanh
Ident = mybir.ActivationFunctionType.Identity


# ---------------------------------------------------------------- walrus fix
# This walrus build supports exactly ONE sync-wait per instruction; Tile
# attaches several. Hoist extras onto same-engine NoOps placed just before.
_ws_counter = [0]


def _split_multi_waits(nc):
    for f in nc.m.functions:
        for bb in f.blocks:
            out = []
            for inst in bb.instructions:
                si = inst.sync_info
                if si is not None and si.on_wait and len(si.on_wait) > 1:
                    waits = list(si.on_wait)
                    for w in waits[:-1]:
                        _ws_counter[0] += 1
                        nop = mybir.InstNoOp(
                            name=f"I-waitsplit-{_ws_counter[0]}",
                            opcode="NoOp",
                            engine=inst.engine,
                            debug=inst.debug,
                            ins=[],
                            outs=[],
                        )
                        nop.sync_info = mybir.SyncInfo(on_wait=[w], on_update=[])
                        out.append(nop)
                    si.on_wait.clear()
                    si.on_wait.append(waits[-1])
                out.append(inst)
            bb.instructions[:] = out


# ---------------------------------------------------------------- planning
def make_plan(text_lengths):
    """Distribute runs across cores. Returns (runs_per_core, r_pc)."""
    L = np.asarray(text_lengths).astype(np.int64)
    runs = []
    for b in range(B):
        lb = int(min(L[b], NCH))
        for n0 in range(0, ((lb + R - 1) // R) * R, R):
            runs.append((b, n0))
    r_pc = -(-len(runs) // NCOREs)              # ceil
    r_pc = -(-r_pc // RPC_CHUNK) * RPC_CHUNK    # pad to multiple of 64
    per_core = [runs[k::NCOREs] for k in range(NCOREs)]  # round-robin
    return per_core, r_pc


# ---------------------------------------------------------------- program
def build_program(r_pc, reps=1):
    """SPMD single-core program for r_pc runs (C = 8*r_pc cols)."""
    f32 = _FP32
    bf = _BF16
    C = R * r_pc
    XCOLS = PB * r_pc                    # XG flat columns
    n512 = C // CW                       # 512-col chunks
    assert r_pc % RPC_CHUNK == 0

    nc = bass.Bass("TRN2", target_bir_lowering=False, debug=False,
                   num_devices=NCOREs)

    def din(name, shape, dtype):
        return nc.declare_dram_parameter(name, list(shape), dtype, isOutput=False)

    tab_in = din("tab", [128, XCOLS], bf)
    gidx_in = din("gidx", [128, XCOLS // 16], _U16)
    whhT_f_in = din("whhT_f", [128, 4 * H], bf)
    whhT_b_in = din("whhT_b", [128, 4 * H], bf)
    wihT_f_in = din("wihT_f", [128, 4 * H], bf)
    wihT_b_in = din("wihT_b", [128, 4 * H], bf)
    bias_f_in = din("bias_f", [128, 4], f32)
    bias_b_in = din("bias_b", [128, 4], f32)
    ident_in = din("ident", [128, 128], bf)
    cmask_in = din("cmask", [128, F, C], _U8)
    zmask_in = din("zmask", [128, XCOLS], f32)
    tailneg_in = din("tailneg", [128, C], _U8)
    pool_out = nc.declare_dram_parameter("pool", [128, 2, r_pc], f32,
                                         isOutput=True)

    def r32(ap):
        return ap.bitcast(_F32R)

    with tile.TileContext(nc) as tc:
        import contextlib
        with contextlib.ExitStack() as ctx:
            const = ctx.enter_context(tc.tile_pool(name="const", bufs=1))
            big = ctx.enter_context(tc.tile_pool(name="big", bufs=2))
            state = ctx.enter_context(tc.tile_pool(name="state", bufs=2))
            work = ctx.enter_context(tc.tile_pool(name="work", bufs=3))
            ps = ctx.enter_context(tc.tile_pool(name="ps", bufs=2, space="PSUM"))

            # ---- constant loads (once) ----
            t_whhT = {}
            t_wihT = {}
            t_bias = {}
            for dirn, w_in, wi_in, b_in in (
                ("f", whhT_f_in, wihT_f_in, bias_f_in),
                ("b", whhT_b_in, wihT_b_in, bias_b_in),
            ):
                t_whhT[dirn] = const.tile([128, 4 * H], bf, tag=f"whhT_{dirn}", name=f"whhT_{dirn}")
                nc.sync.dma_start(out=t_whhT[dirn][:], in_=w_in[:])
                t_wihT[dirn] = const.tile([128, 4 * H], bf, tag=f"wihT_{dirn}", name=f"wihT_{dirn}")
                nc.sync.dma_start(out=t_wihT[dirn][:], in_=wi_in[:])
                t_bias[dirn] = const.tile([128, 4], f32, tag=f"bias_{dirn}", name=f"bias_{dirn}")
                nc.sync.dma_start(out=t_bias[dirn][:], in_=b_in[:])
            t_ident = const.tile([128, 128], bf, tag="ident", name="ident")
            nc.sync.dma_start(out=t_ident[:], in_=ident_in[:])

            # PE warmup: ramp the tensor engine to full clock while the
            # input DMAs land (outputs are never read).
            t_warm = const.tile([128, CW], bf, tag="warm", name="warm")
            nc.vector.memset(t_warm[:], 0.0)
            psw = ps.tile([128, 4, CW], f32, tag="ps", name="psw")
            for w in range(12):
                nc.tensor.matmul(psw[:, w % 4, :], t_whhT["f"][:, 0:H],
                                 t_warm[:], start=True, stop=True)

            def make_prologue(rep):
                """Per-rep input load + XG production, split into parts
                that can be interleaved with the previous rep's steps."""
                hd = {}
                parts = []

                def p_dma():
                    hd["tab"] = big.tile([128, XCOLS], bf, tag="tab", name="tab")
                    nc.sync.dma_start(out=hd["tab"][:], in_=tab_in[:])
                    hd["gidx"] = big.tile([128, XCOLS // 16], _U16, tag="gidx", name="gidx")
                    nc.sync.dma_start(out=hd["gidx"][:], in_=gidx_in[:])
                    hd["zmask"] = big.tile([128, XCOLS], f32, tag="zmask", name="zmask")
                    nc.sync.dma_start(out=hd["zmask"][:], in_=zmask_in[:])
                    hd["cmask"] = big.tile([128, F, C], _U8, tag="cmask", name="cmask")
                    nc.sync.dma_start(out=hd["cmask"][:], in_=cmask_in[:])
                    hd["tailneg"] = big.tile([128, C], _U8, tag="tailneg", name="tailneg")
                    nc.sync.dma_start(out=hd["tailneg"][:], in_=tailneg_in[:])
                    hd["xT32"] = big.tile([128, XCOLS, 1], bf, tag="xT32", name="xT32")
                    hd["XG_f"] = big.tile([128, 4, XCOLS], bf, tag="XG_f", name="XG_f")
                    hd["XG_b"] = big.tile([128, 4, XCOLS], bf, tag="XG_b", name="XG_b")
                parts.append(p_dma)

                for s0g in range(0, XCOLS, CW):
                    def p_gather(s0=s0g):
                        s1 = min(s0 + CW, XCOLS)
                        nc.gpsimd.indirect_copy(
                            hd["xT32"][:, s0:s1, :], hd["tab"][:],
                            hd["gidx"][:, s0 // 16:(s1 + 15) // 16],
                            i_know_ap_gather_is_preferred=True)
                    parts.append(p_gather)

                for dirn in ("f", "b"):
                    for s0x in range(0, XCOLS, CW):
                        def p_xg(dirn=dirn, s0=s0x):
                            s1 = min(s0 + CW, XCOLS)
                            xT = hd["xT32"][:, :, 0]
                            psx = ps.tile([128, 4, CW], f32, tag="ps", name="ps")
                            for g in range(4):
                                nc.tensor.matmul(
                                    psx[:, g, 0:s1 - s0],
                                    t_wihT[dirn][:, g * H:(g + 1) * H],
                                    xT[:, s0:s1],
                                    start=True, stop=True)
                            for g in range(4):
                                if dirn == "f":
                                    nc.vector.tensor_scalar_add(
                                        hd["XG_f"][:, g, s0:s1],
                                        psx[:, g, 0:s1 - s0],
                                        t_bias[dirn][:, g:g + 1])
                                else:
                                    nc.vector.scalar_tensor_tensor(
                                        hd["XG_b"][:, g, s0:s1],
                                        psx[:, g, 0:s1 - s0],
                                        t_bias[dirn][:, g:g + 1],
                                        hd["zmask"][:, s0:s1],
                                        op0=mybir.AluOpType.add,
                                        op1=mybir.AluOpType.mult)
                        parts.append(p_xg)

                def p_state():
                    for dirn in ("f", "b"):
                        hd[f"h_{dirn}"] = state.tile([128, C], bf, tag=f"h_{dirn}", name=f"h_{dirn}")
                        nc.vector.memset(hd[f"h_{dirn}"][:], 0.0)
                        hd[f"c_{dirn}"] = state.tile([128, C], bf, tag=f"c_{dirn}", name=f"c_{dirn}")
                        nc.vector.memset(hd[f"c_{dirn}"][:], 0.0)
                    hd["hcap"] = state.tile([128, C], bf, tag="hcap", name="hcap")
                    nc.vector.memset(hd["hcap"][:], NEG)
                parts.append(p_state)
                return parts, hd

            def emit_rep(hd, inject):
                """One rep's 16x2 steps + epilogue; inject[t] = closures to
                emit after step t (next rep's prologue parts)."""
                xg_v = {d: hd[f"XG_{d}"][:].rearrange("p g (r s) -> p g r s", s=PB)
                        for d in ("f", "b")}
                for t in range(F):
                    for dirn in ("f", "b"):
                        off = t if dirn == "f" else (F - 1 - t)
                        h, c = hd[f"h_{dirn}"], hd[f"c_{dirn}"]
                        ifo = work.tile([128, 3, C], bf, tag="ifo", name="ifo")
                        gct = work.tile([128, C], bf, tag="gct", name="gct")
                        for cc in range(n512):
                            s0 = cc * CW
                            s1 = s0 + CW
                            r0 = cc * RPC_CHUNK
                            r1 = r0 + RPC_CHUNK
                            psg = ps.tile([128, 4, CW], f32, tag="ps", name="ps")
                            for g in (0, 1, 2, 3):
                                nc.tensor.matmul(
                                    psg[:, g, :],
                                    t_whhT[dirn][:, g * H:(g + 1) * H],
                                    h[:, s0:s1],
                                    start=True, stop=False)
                                nc.tensor.matmul(
                                    psg[:, g, :],
                                    t_ident[:],
                                    xg_v[dirn][:, g, r0:r1, off:off + R],
                                    start=False, stop=True)
                            # gates: sigmoid(i,f,o) after 6 mms, tanh(g)
                            # after 8
                            nc.scalar.activation(
                                ifo[:, :, s0:s1], psg[:, 0:3, :], Sig)
                            nc.scalar.activation(
                                gct[:, s0:s1], psg[:, 3, :], Tanh)
                        # c = f*c + i*g
                        tm1 = work.tile([128, C], bf, tag="tm1", name="tm1")
                        nc.vector.tensor_mul(tm1[:], ifo[:, 1, :], c[:])
                        tm2 = work.tile([128, C], bf, tag="tm2", name="tm2")
                        nc.gpsimd.tensor_mul(tm2[:], ifo[:, 0, :], gct[:])
                        nc.vector.tensor_add(c[:], tm1[:], tm2[:])
                        tct = work.tile([128, C], bf, tag="tct", name="tct")
                        nc.scalar.activation(tct[:], c[:], Tanh)
                        # h = o * tanh(c)
                        nc.vector.tensor_mul(h[:], ifo[:, 2, :], tct[:])
                        if dirn == "f":
                            nc.vector.copy_predicated(
                                hd["hcap"][:], hd["cmask"][:, t, :], h[:])
                    for fn in inject.get(t, []):
                        fn()

                # ---- epilogue ----
                t_negc = work.tile([128, C], bf, tag="tm1", name="negc")
                nc.vector.memset(t_negc[:], NEG)
                nc.vector.copy_predicated(hd["h_b"][:], hd["tailneg"][:], t_negc[:])

                t_pool = work.tile([128, 2, r_pc], f32, tag="pool", name="pool")
                nc.vector.tensor_reduce(
                    t_pool[:, 0, :],
                    hd["hcap"][:].rearrange("p (r e) -> p r e", e=R),
                    axis=mybir.AxisListType.X, op=mybir.AluOpType.max)
                nc.vector.tensor_reduce(
                    t_pool[:, 1, :],
                    hd["h_b"][:].rearrange("p (r e) -> p r e", e=R),
                    axis=mybir.AxisListType.X, op=mybir.AluOpType.max)
                nc.sync.dma_start(out=pool_out[:], in_=t_pool[:])

            parts, hd = make_prologue(0)
            for p in parts:
                p()
            for rep in range(reps):
                if rep + 1 < reps:
                    nparts, nhd = make_prologue(rep + 1)
                    # spread next-rep prologue over steps 4..4+len-1
                    inject = {4 + i: [p] for i, p in enumerate(nparts)}
                else:
                    nhd = None
                    inject = {}
                emit_rep(hd, inject)
                hd = nhd

    return nc


# ---------------------------------------------------------------- host prep
def _bf16(x):
    return np.asarray(x, dtype=mybir.dt.np(_BF16))


def wT(w, dt=np.float32):
    """[4H, X] -> [X, 4H] device gate order (i, f, g, o)."""
    t = np.ascontiguousarray(np.asarray(w, dtype=np.float32).T)
    return np.concatenate(
        [t[:, g * H:(g + 1) * H] for g in GPERM], axis=1).astype(dt)


def bcols(b):
    b = np.asarray(b, dtype=np.float32)
    return np.stack([b[g * H:(g + 1) * H] for g in GPERM], axis=1)  # [128,4]


def wrap_idx(idx, xcols):
    n = len(idx)
    cols = xcols // 16
    pad = np.zeros(cols * 16, dtype=np.uint16)
    pad[:n] = idx
    return np.tile(pad.reshape(cols, 16).T, (8, 1))  # [128, cols]


def host_inputs(text, text_lengths, emb, w_ih_f, w_hh_f, b_f,
                w_ih_b, w_hh_b, b_b):
    """Build per-core input dicts + the plan."""
    text = np.asarray(text).astype(np.int64)            # [S, B]
    L = np.asarray(text_lengths).astype(np.int64)       # [B]
    emb = np.asarray(emb, dtype=np.float32)

    per_core_runs, r_pc = make_plan(L)
    C = R * r_pc
    XCOLS = PB * r_pc

    bfnp = mybir.dt.np(_BF16)
    common = dict(
        whhT_f=wT(w_hh_f, bfnp), whhT_b=wT(w_hh_b, bfnp),
        wihT_f=wT(w_ih_f, bfnp), wihT_b=wT(w_ih_b, bfnp),
        bias_f=bcols(b_f), bias_b=bcols(b_b),
        ident=np.eye(128, dtype=np.float32).astype(bfnp),
    )

    in_maps = []
    for k in range(NCOREs):
        runs = per_core_runs[k]
        # slot positions [r_pc, PB] and their batch
        pos = np.zeros((r_pc, PB), dtype=np.int64)
        bat = np.zeros((r_pc,), dtype=np.int64)
        valid = np.zeros((r_pc,), dtype=bool)
        for j, (b, n0) in enumerate(runs):
            pos[j] = np.minimum(n0 + np.arange(PB), S - 1)
            bat[j] = b
            valid[j] = True
        toks = text[pos.ravel(), np.repeat(np.where(valid, bat, 0), PB)]
        uniq, ranks = np.unique(toks, return_inverse=True)
        assert len(uniq) <= XCOLS
        tab = np.zeros((128, XCOLS), dtype=np.float32)
        tab[:, :len(uniq)] = emb[uniq].T
        gidx = wrap_idx(ranks.astype(np.uint16), XCOLS)

        # zmask: position < L_b (per slot)
        zm = (pos < L[np.where(valid, bat, 0)][:, None]) & valid[:, None]
        zmask = zm.astype(np.float32).reshape(1, XCOLS)

        # per-column chunk index and capture length
        ncol = np.where(valid[:, None], pos[:, :R], NCH)  # [r_pc, R] chunk n
        Lb = L[np.where(valid, bat, 0)][:, None]
        l_eff = np.clip(Lb - ncol, None, F)               # <=0 -> tail
        is_tail = (ncol >= np.minimum(Lb, NCH)) | ~valid[:, None]
        cmask = np.zeros((F, r_pc, R), dtype=np.float32)
        for t in range(F):
            cmask[t] = ((l_eff == t + 1) & ~is_tail).astype(np.float32)
        tailneg = is_tail.astype(np.float32).reshape(1, C)

        m = dict(common)
        m["tab"] = _bf16(tab)
        m["gidx"] = gidx
        m["zmask"] = np.broadcast_to(zmask, (128, XCOLS)).copy()
        m["cmask"] = np.broadcast_to(
            cmask.reshape(1, F, C), (128, F, C)).astype(np.uint8)
        m["tailneg"] = np.broadcast_to(tailneg, (128, C)).astype(np.uint8)
        in_maps.append(m)
    return in_maps, per_core_runs, r_pc


# ---------------------------------------------------------------- host finish
def _cell1(xg):
    """Single-step LSTM cell from zero state; xg: [..., 4H] ref gate order."""
    i = 1.0 / (1.0 + np.exp(-xg[..., :H]))
    g = np.tanh(xg[..., 2 * H:3 * H])
    o = 1.0 / (1.0 + np.exp(-xg[..., 3 * H:]))
    c = i * g
    return o * np.tanh(c)


def host_finish(pools, per_core_runs, text, text_lengths, emb,
                w_ih_f, b_f, w_ih_b, b_b, w_fc, b_fc):
    text = np.asarray(text).astype(np.int64)
    L = np.asarray(text_lengths).astype(np.int64)
    emb = np.asarray(emb, dtype=np.float32)

    hf = np.full((128, B), NEG, dtype=np.float32)
    hb = np.full((128, B), NEG, dtype=np.float32)
    for k in range(NCOREs):
        p = np.asarray(pools[k], dtype=np.float32)      # [128, 2, r_pc]
        for j, (b, n0) in enumerate(per_core_runs[k]):
            np.maximum(hf[:, b], p[:, 0, j], out=hf[:, b])
            np.maximum(hb[:, b], p[:, 1, j], out=hb[:, b])

    # tail chunks: n in [L_b, 241) -> single-step cell of position n.
    Lmin = int(L.min())
    if Lmin < NCH:
        for b in range(B):
            lb = int(L[b])
            if lb >= NCH:
                continue
            nsb = np.arange(lb, NCH)
            x = emb[text[nsb, b]]                        # [nt, E]
            xg_f = x @ np.asarray(w_ih_f, np.float32).T + np.asarray(b_f, np.float32)
            xg_b = x @ np.asarray(w_ih_b, np.float32).T + np.asarray(b_b, np.float32)
            vf = _cell1(xg_f).max(axis=0)                # [H]
            vb = _cell1(xg_b).max(axis=0)
            np.maximum(hf[:, b], vf, out=hf[:, b])
            np.maximum(hb[:, b], vb, out=hb[:, b])

    hid = np.concatenate([hf.T, hb.T], axis=1)           # [B, 2H]
    w_fc = np.asarray(w_fc, dtype=np.float32)
    b_fc = np.asarray(b_fc, dtype=np.float32)
    return (hid @ w_fc.T + b_fc).astype(np.float32)


# ---------------------------------------------------------------- runner
_CACHE = {}


def get_runner(r_pc, reps=1):
    key = (r_pc, reps)
    if key not in _CACHE:
        nc = build_program(r_pc, reps=reps)
        _split_multi_waits(nc)
        _CACHE[key] = nc
    return _CACHE[key]


def run_on_device(nc, in_maps):
    res = bass2jax.run_bass_via_pjrt(nc, in_maps, n_cores=NCOREs)
    return [r["pool"] for r in res]


def kernel(text, text_lengths, emb, w_ih_f, w_hh_f, b_f,
           w_ih_b, w_hh_b, b_b, w_fc, b_fc):
    in_maps, per_core_runs, r_pc = host_inputs(
        text, text_lengths, emb, w_ih_f, w_hh_f, b_f, w_ih_b, w_hh_b, b_b)
    nc = get_runner(r_pc, reps=1)
    pools = run_on_device(nc, in_maps)
    return host_finish(pools, per_core_runs, text, text_lengths, emb,
                       w_ih_f, b_f, w_ih_b, b_b, w_fc, b_fc)


# revision 34
# speedup vs baseline: 3.3622x; 1.2532x over previous
"""Trainium2 Bass kernel for the sliding-window bidirectional-LSTM "CNN".

Self-contained: hardcodes shapes for the nn_CNN problem
(S=256, B=32, F=16, H=128, E=128, OUT=5, V=50257, 8 cores).

Run-packed strategy (v2): only chunks n < L_b are computed on device.
  - a "run" = R=8 consecutive chunks of ONE batch b -> 8 columns.
  - runs tile [0, ceil(min(L_b,241)/8)*8) per batch; all runs distributed
    over 8 cores; padded to 64 runs/core -> C=512 columns per core
    (PSUM: each gate x 512 cols = exactly one 2KB bank).
  - XG = W_ih.x + b precomputed per run over R+15=23 positions
    ([128, 4, r_pc, 23] bf16); the step-t matmul rhs is the 2-level AP
    [:, g, :, t:t+8] (forward) or [:, g, :, 15-t:23-t] (backward).
  - bias folded at XG production; backward XG additionally multiplied by
    zmask (positions >= L_b -> 0) via one fused scalar_tensor_tensor.
  - forward: capture h at t == l-1 via per-step copy_predicated masks
    (full cols capture at t=15, edge cols earlier). bf16 elementwise.
  - backward: runs 16 steps positions 15..0; zmask keeps state 0 until
    the valid suffix begins; final h is the answer; tail/pad columns
    forced to -1e30 before pooling.
  - per-run max-pool on device -> [128, 2, r_pc] partials; host maps
    runs->batches, adds the tail-chunk contribution (chunks n >= L_b are
    single-step cells of position n only -- batch-independent, done on
    host as a suffix-max over positions), and applies the 5-dim FC.
"""

import numpy as np

import concourse.bass as bass
import concourse.tile as tile
import concourse.mybir as mybir
from concourse import bass2jax

# ---------------------------------------------------------------- constants
S, B, F, H, E, OUT, V = 256, 32, 16, 128, 128, 5, 50257
NCOREs = 8
NCH = S - F + 1      # 241 chunks total
R = 8                # chunks per run
PB = R + F - 1       # 23 positions per run block
GPERM = [0, 1, 3, 2]  # device gate order (i, f, o, g) <- reference (i, f, g, o)
NEG = -1.0e30
CW = 512             # columns per matmul chunk (one PSUM bank per gate)
RPC_CHUNK = CW // R  # 64 runs per 512-col chunk

_FP32 = mybir.dt.float32
_F32R = mybir.dt.float32r
_BF16 = mybir.dt.bfloat16
_U16 = mybir.dt.uint16
_U8 = mybir.dt.uint8

Sig = mybir.ActivationFunctionType.Sigmoid
Tanh = mybir.ActivationFunctionType.Tanh
Ident = mybir.ActivationFunctionType.Identity


# ---------------------------------------------------------------- walrus fix
# This walrus build supports exactly ONE sync-wait per instruction; Tile
# attaches several. Hoist extras onto same-engine NoOps placed just before.
_ws_counter = [0]


def _split_multi_waits(nc):
    for f in nc.m.functions:
        for bb in f.blocks:
            out = []
            for inst in bb.instructions:
                si = inst.sync_info
                if si is not None and si.on_wait and len(si.on_wait) > 1:
                    waits = list(si.on_wait)
                    for w in waits[:-1]:
                        _ws_counter[0] += 1
                        nop = mybir.InstNoOp(
                            name=f"I-waitsplit-{_ws_counter[0]}",
                            opcode="NoOp",
                            engine=inst.engine,
                            debug=inst.debug,
                            ins=[],
                            outs=[],
                        )
                        nop.sync_info = mybir.SyncInfo(on_wait=[w], on_update=[])
                        out.append(nop)
                    si.on_wait.clear()
                    si.on_wait.append(waits[-1])
                out.append(inst)
            bb.instructions[:] = out


# ---------------------------------------------------------------- planning
def make_plan(text_lengths):
    """Distribute runs across cores. Returns (runs_per_core, r_pc)."""
    L = np.asarray(text_lengths).astype(np.int64)
    runs = []
    for b in range(B):
        lb = int(min(L[b], NCH))
        for n0 in range(0, ((lb + R - 1) // R) * R, R):
            runs.append((b, n0))
    r_pc = -(-len(runs) // NCOREs)              # ceil
    r_pc = -(-r_pc // RPC_CHUNK) * RPC_CHUNK    # pad to multiple of 64
    per_core = [runs[k::NCOREs] for k in range(NCOREs)]  # round-robin
    return per_core, r_pc


# ---------------------------------------------------------------- program
def build_program(r_pc, reps=1):
    """SPMD single-core program for r_pc runs (C = 8*r_pc cols)."""
    f32 = _FP32
    bf = _BF16
    C = R * r_pc
    XCOLS = PB * r_pc                    # XG flat columns
    n512 = C // CW                       # 512-col chunks
    assert r_pc % RPC_CHUNK == 0

    nc = bass.Bass("TRN2", target_bir_lowering=False, debug=False,
                   num_devices=NCOREs)

    def din(name, shape, dtype):
        return nc.declare_dram_parameter(name, list(shape), dtype, isOutput=False)

    tab_in = din("tab", [128, XCOLS], bf)
    gidx_in = din("gidx", [128, XCOLS // 16], _U16)
    whhT_f_in = din("whhT_f", [128, 4 * H], bf)
    whhT_b_in = din("whhT_b", [128, 4 * H], bf)
    wihT_f_in = din("wihT_f", [128, 4 * H], bf)
    wihT_b_in = din("wihT_b", [128, 4 * H], bf)
    bias_f_in = din("bias_f", [128, 4], f32)
    bias_b_in = din("bias_b", [128, 4], f32)
    ident_in = din("ident", [128, 128], bf)
    cmask_in = din("cmask", [128, F, C], _U8)
    zmask_in = din("zmask", [128, XCOLS], f32)
    tailneg_in = din("tailneg", [128, C], _U8)
    pool_out = nc.declare_dram_parameter("pool", [128, 2, r_pc], f32,
                                         isOutput=True)

    def r32(ap):
        return ap.bitcast(_F32R)

    with tile.TileContext(nc) as tc:
        import contextlib
        with contextlib.ExitStack() as ctx:
            const = ctx.enter_context(tc.tile_pool(name="const", bufs=1))
            big = ctx.enter_context(tc.tile_pool(name="big", bufs=2))
            state = ctx.enter_context(tc.tile_pool(name="state", bufs=2))
            work = ctx.enter_context(tc.tile_pool(name="work", bufs=4))
            ps = ctx.enter_context(tc.tile_pool(name="ps", bufs=2, space="PSUM"))

            # ---- constant loads (once) ----
            t_whhT = {}
            t_wihT = {}
            t_bias = {}
            for dirn, w_in, wi_in, b_in in (
                ("f", whhT_f_in, wihT_f_in, bias_f_in),
                ("b", whhT_b_in, wihT_b_in, bias_b_in),
            ):
                t_whhT[dirn] = const.tile([128, 4 * H], bf, tag=f"whhT_{dirn}", name=f"whhT_{dirn}")
                nc.sync.dma_start(out=t_whhT[dirn][:], in_=w_in[:])
                t_wihT[dirn] = const.tile([128, 4 * H], bf, tag=f"wihT_{dirn}", name=f"wihT_{dirn}")
                nc.sync.dma_start(out=t_wihT[dirn][:], in_=wi_in[:])
                t_bias[dirn] = const.tile([128, 4], f32, tag=f"bias_{dirn}", name=f"bias_{dirn}")
                nc.sync.dma_start(out=t_bias[dirn][:], in_=b_in[:])
            t_ident = const.tile([128, 128], bf, tag="ident", name="ident")
            nc.sync.dma_start(out=t_ident[:], in_=ident_in[:])

            # PE warmup: ramp the tensor engine to full clock while the
            # input DMAs land (outputs are never read).
            t_warm = const.tile([128, CW], bf, tag="warm", name="warm")
            nc.vector.memset(t_warm[:], 0.0)
            psw = ps.tile([128, 4, CW], f32, tag="ps", name="psw")
            for w in range(12):
                nc.tensor.matmul(psw[:, w % 4, :], t_whhT["f"][:, 0:H],
                                 t_warm[:], start=True, stop=True)

            def make_prologue(rep):
                """Per-rep input load + XG production, split into parts
                that can be interleaved with the previous rep's steps."""
                hd = {}
                parts = []

                def p_dma():
                    hd["tab"] = big.tile([128, XCOLS], bf, tag="tab", name="tab")
                    nc.sync.dma_start(out=hd["tab"][:], in_=tab_in[:])
                    hd["gidx"] = big.tile([128, XCOLS // 16], _U16, tag="gidx", name="gidx")
                    nc.sync.dma_start(out=hd["gidx"][:], in_=gidx_in[:])
                    hd["zmask"] = big.tile([128, XCOLS], f32, tag="zmask", name="zmask")
                    nc.sync.dma_start(out=hd["zmask"][:], in_=zmask_in[:])
                    hd["cmask"] = big.tile([128, F, C], _U8, tag="cmask", name="cmask")
                    nc.sync.dma_start(out=hd["cmask"][:], in_=cmask_in[:])
                    hd["tailneg"] = big.tile([128, C], _U8, tag="tailneg", name="tailneg")
                    nc.sync.dma_start(out=hd["tailneg"][:], in_=tailneg_in[:])
                    hd["xT32"] = big.tile([128, XCOLS, 1], bf, tag="xT32", name="xT32")
                    hd["XG_f"] = big.tile([128, 4, XCOLS], bf, tag="XG_f", name="XG_f")
                    hd["XG_b"] = big.tile([128, 4, XCOLS], bf, tag="XG_b", name="XG_b")
                parts.append(p_dma)

                for s0g in range(0, XCOLS, CW):
                    def p_gather(s0=s0g):
                        s1 = min(s0 + CW, XCOLS)
                        nc.gpsimd.indirect_copy(
                            hd["xT32"][:, s0:s1, :], hd["tab"][:],
                            hd["gidx"][:, s0 // 16:(s1 + 15) // 16],
                            i_know_ap_gather_is_preferred=True)
                    parts.append(p_gather)

                for dirn in ("f", "b"):
                    for s0x in range(0, XCOLS, CW):
                        def p_xg(dirn=dirn, s0=s0x):
                            s1 = min(s0 + CW, XCOLS)
                            xT = hd["xT32"][:, :, 0]
                            psx = ps.tile([128, 4, CW], f32, tag="ps", name="ps")
                            for g in range(4):
                                nc.tensor.matmul(
                                    psx[:, g, 0:s1 - s0],
                                    t_wihT[dirn][:, g * H:(g + 1) * H],
                                    xT[:, s0:s1],
                                    start=True, stop=True)
                            for g in range(4):
                                if dirn == "f":
                                    nc.vector.tensor_scalar_add(
                                        hd["XG_f"][:, g, s0:s1],
                                        psx[:, g, 0:s1 - s0],
                                        t_bias[dirn][:, g:g + 1])
                                else:
                                    nc.vector.scalar_tensor_tensor(
                                        hd["XG_b"][:, g, s0:s1],
                                        psx[:, g, 0:s1 - s0],
                                        t_bias[dirn][:, g:g + 1],
                                        hd["zmask"][:, s0:s1],
                                        op0=mybir.AluOpType.add,
                                        op1=mybir.AluOpType.mult)
                        parts.append(p_xg)

                def p_state():
                    for dirn in ("f", "b"):
                        hd[f"h_{dirn}"] = state.tile([128, C], bf, tag=f"h_{dirn}", name=f"h_{dirn}")
                        nc.vector.memset(hd[f"h_{dirn}"][:], 0.0)
                        hd[f"c_{dirn}"] = state.tile([128, C], bf, tag=f"c_{dirn}", name=f"c_{dirn}")
                        nc.vector.memset(hd[f"c_{dirn}"][:], 0.0)
                    hd["hcap"] = state.tile([128, C], bf, tag="hcap", name="hcap")
                    nc.vector.memset(hd["hcap"][:], NEG)
                parts.append(p_state)
                return parts, hd

            def emit_rep(hd, inject):
                """One rep's 16x2 steps + epilogue; inject[t] = closures to
                emit after step t (next rep's prologue parts)."""
                xg_v = {d: hd[f"XG_{d}"][:].rearrange("p g (r s) -> p g r s", s=PB)
                        for d in ("f", "b")}
                for t in range(F):
                    for dirn in ("f", "b"):
                        off = t if dirn == "f" else (F - 1 - t)
                        h, c = hd[f"h_{dirn}"], hd[f"c_{dirn}"]
                        ifo = work.tile([128, 3, C], bf, tag="ifo", name="ifo")
                        gct = work.tile([128, C], bf, tag="gct", name="gct")
                        for cc in range(n512):
                            s0 = cc * CW
                            s1 = s0 + CW
                            r0 = cc * RPC_CHUNK
                            r1 = r0 + RPC_CHUNK
                            psg = ps.tile([128, 4, CW], f32, tag="ps", name="ps")
                            for g in (0, 1, 2, 3):
                                nc.tensor.matmul(
                                    psg[:, g, :],
                                    t_whhT[dirn][:, g * H:(g + 1) * H],
                                    h[:, s0:s1],
                                    start=True, stop=False)
                                nc.tensor.matmul(
                                    psg[:, g, :],
                                    t_ident[:],
                                    xg_v[dirn][:, g, r0:r1, off:off + R],
                                    start=False, stop=True)
                            # gates: sigmoid(i,f,o) after 6 mms, tanh(g)
                            # after 8
                            nc.scalar.activation(
                                ifo[:, :, s0:s1], psg[:, 0:3, :], Sig)
                            nc.scalar.activation(
                                gct[:, s0:s1], psg[:, 3, :], Tanh)
                        # c = f*c + i*g
                        tm1 = work.tile([128, C], bf, tag="tm1", name="tm1")
                        nc.vector.tensor_mul(tm1[:], ifo[:, 1, :], c[:])
                        tm2 = work.tile([128, C], bf, tag="tm2", name="tm2")
                        nc.gpsimd.tensor_mul(tm2[:], ifo[:, 0, :], gct[:])
                        nc.vector.tensor_add(c[:], tm1[:], tm2[:])
                        tct = work.tile([128, C], bf, tag="tct", name="tct")
                        nc.scalar.activation(tct[:], c[:], Tanh)
                        # h = o * tanh(c)
                        nc.vector.tensor_mul(h[:], ifo[:, 2, :], tct[:])
                        if dirn == "f":
                            nc.vector.copy_predicated(
                                hd["hcap"][:], hd["cmask"][:, t, :], h[:])
                    for fn in inject.get(t, []):
                        fn()

                # ---- epilogue ----
                t_negc = work.tile([128, C], bf, tag="tm1", name="negc")
                nc.vector.memset(t_negc[:], NEG)
                nc.vector.copy_predicated(hd["h_b"][:], hd["tailneg"][:], t_negc[:])

                t_pool = work.tile([128, 2, r_pc], f32, tag="pool", name="pool")
                nc.vector.tensor_reduce(
                    t_pool[:, 0, :],
                    hd["hcap"][:].rearrange("p (r e) -> p r e", e=R),
                    axis=mybir.AxisListType.X, op=mybir.AluOpType.max)
                nc.vector.tensor_reduce(
                    t_pool[:, 1, :],
                    hd["h_b"][:].rearrange("p (r e) -> p r e", e=R),
                    axis=mybir.AxisListType.X, op=mybir.AluOpType.max)
                nc.sync.dma_start(out=pool_out[:], in_=t_pool[:])

            parts, hd = make_prologue(0)
            for p in parts:
                p()
            for rep in range(reps):
                if rep + 1 < reps:
                    nparts, nhd = make_prologue(rep + 1)
                    # spread next-rep prologue over steps 4..4+len-1
                    inject = {4 + i: [p] for i, p in enumerate(nparts)}
                else:
                    nhd = None
                    inject = {}
                emit_rep(hd, inject)
                hd = nhd

    return nc


# ---------------------------------------------------------------- host prep
def _bf16(x):
    return np.asarray(x, dtype=mybir.dt.np(_BF16))


def wT(w, dt=np.float32):
    """[4H, X] -> [X, 4H] device gate order (i, f, g, o)."""
    t = np.ascontiguousarray(np.asarray(w, dtype=np.float32).T)
    return np.concatenate(
        [t[:, g * H:(g + 1) * H] for g in GPERM], axis=1).astype(dt)


def bcols(b):
    b = np.asarray(b, dtype=np.float32)
    return np.stack([b[g * H:(g + 1) * H] for g in GPERM], axis=1)  # [128,4]


def wrap_idx(idx, xcols):
    n = len(idx)
    cols = xcols // 16
    pad = np.zeros(cols * 16, dtype=np.uint16)
    pad[:n] = idx
    return np.tile(pad.reshape(cols, 16).T, (8, 1))  # [128, cols]


def host_inputs(text, text_lengths, emb, w_ih_f, w_hh_f, b_f,
                w_ih_b, w_hh_b, b_b):
    """Build per-core input dicts + the plan."""
    text = np.asarray(text).astype(np.int64)            # [S, B]
    L = np.asarray(text_lengths).astype(np.int64)       # [B]
    emb = np.asarray(emb, dtype=np.float32)

    per_core_runs, r_pc = make_plan(L)
    C = R * r_pc
    XCOLS = PB * r_pc

    bfnp = mybir.dt.np(_BF16)
    common = dict(
        whhT_f=wT(w_hh_f, bfnp), whhT_b=wT(w_hh_b, bfnp),
        wihT_f=wT(w_ih_f, bfnp), wihT_b=wT(w_ih_b, bfnp),
        bias_f=bcols(b_f), bias_b=bcols(b_b),
        ident=np.eye(128, dtype=np.float32).astype(bfnp),
    )

    in_maps = []
    for k in range(NCOREs):
        runs = per_core_runs[k]
        # slot positions [r_pc, PB] and their batch
        pos = np.zeros((r_pc, PB), dtype=np.int64)
        bat = np.zeros((r_pc,), dtype=np.int64)
        valid = np.zeros((r_pc,), dtype=bool)
        for j, (b, n0) in enumerate(runs):
            pos[j] = np.minimum(n0 + np.arange(PB), S - 1)
            bat[j] = b
            valid[j] = True
        toks = text[pos.ravel(), np.repeat(np.where(valid, bat, 0), PB)]
        uniq, ranks = np.unique(toks, return_inverse=True)
        assert len(uniq) <= XCOLS
        tab = np.zeros((128, XCOLS), dtype=np.float32)
        tab[:, :len(uniq)] = emb[uniq].T
        gidx = wrap_idx(ranks.astype(np.uint16), XCOLS)

        # zmask: position < L_b (per slot)
        zm = (pos < L[np.where(valid, bat, 0)][:, None]) & valid[:, None]
        zmask = zm.astype(np.float32).reshape(1, XCOLS)

        # per-column chunk index and capture length
        ncol = np.where(valid[:, None], pos[:, :R], NCH)  # [r_pc, R] chunk n
        Lb = L[np.where(valid, bat, 0)][:, None]
        l_eff = np.clip(Lb - ncol, None, F)               # <=0 -> tail
        is_tail = (ncol >= np.minimum(Lb, NCH)) | ~valid[:, None]
        cmask = np.zeros((F, r_pc, R), dtype=np.float32)
        for t in range(F):
            cmask[t] = ((l_eff == t + 1) & ~is_tail).astype(np.float32)
        tailneg = is_tail.astype(np.float32).reshape(1, C)

        m = dict(common)
        m["tab"] = _bf16(tab)
        m["gidx"] = gidx
        m["zmask"] = np.broadcast_to(zmask, (128, XCOLS)).copy()
        m["cmask"] = np.broadcast_to(
            cmask.reshape(1, F, C), (128, F, C)).astype(np.uint8)
        m["tailneg"] = np.broadcast_to(tailneg, (128, C)).astype(np.uint8)
        in_maps.append(m)
    return in_maps, per_core_runs, r_pc


# ---------------------------------------------------------------- host finish
def _cell1(xg):
    """Single-step LSTM cell from zero state; xg: [..., 4H] ref gate order."""
    i = 1.0 / (1.0 + np.exp(-xg[..., :H]))
    g = np.tanh(xg[..., 2 * H:3 * H])
    o = 1.0 / (1.0 + np.exp(-xg[..., 3 * H:]))
    c = i * g
    return o * np.tanh(c)


def host_finish(pools, per_core_runs, text, text_lengths, emb,
                w_ih_f, b_f, w_ih_b, b_b, w_fc, b_fc):
    text = np.asarray(text).astype(np.int64)
    L = np.asarray(text_lengths).astype(np.int64)
    emb = np.asarray(emb, dtype=np.float32)

    hf = np.full((128, B), NEG, dtype=np.float32)
    hb = np.full((128, B), NEG, dtype=np.float32)
    for k in range(NCOREs):
        p = np.asarray(pools[k], dtype=np.float32)      # [128, 2, r_pc]
        for j, (b, n0) in enumerate(per_core_runs[k]):
            np.maximum(hf[:, b], p[:, 0, j], out=hf[:, b])
            np.maximum(hb[:, b], p[:, 1, j], out=hb[:, b])

    # tail chunks: n in [L_b, 241) -> single-step cell of position n.
    Lmin = int(L.min())
    if Lmin < NCH:
        for b in range(B):
            lb = int(L[b])
            if lb >= NCH:
                continue
            nsb = np.arange(lb, NCH)
            x = emb[text[nsb, b]]                        # [nt, E]
            xg_f = x @ np.asarray(w_ih_f, np.float32).T + np.asarray(b_f, np.float32)
            xg_b = x @ np.asarray(w_ih_b, np.float32).T + np.asarray(b_b, np.float32)
            vf = _cell1(xg_f).max(axis=0)                # [H]
            vb = _cell1(xg_b).max(axis=0)
            np.maximum(hf[:, b], vf, out=hf[:, b])
            np.maximum(hb[:, b], vb, out=hb[:, b])

    hid = np.concatenate([hf.T, hb.T], axis=1)           # [B, 2H]
    w_fc = np.asarray(w_fc, dtype=np.float32)
    b_fc = np.asarray(b_fc, dtype=np.float32)
    return (hid @ w_fc.T + b_fc).astype(np.float32)


# ---------------------------------------------------------------- runner
_CACHE = {}


def get_runner(r_pc, reps=1):
    key = (r_pc, reps)
    if key not in _CACHE:
        nc = build_program(r_pc, reps=reps)
        _split_multi_waits(nc)
        _CACHE[key] = nc
    return _CACHE[key]


def run_on_device(nc, in_maps):
    res = bass2jax.run_bass_via_pjrt(nc, in_maps, n_cores=NCOREs)
    return [r["pool"] for r in res]


def kernel(text, text_lengths, emb, w_ih_f, w_hh_f, b_f,
           w_ih_b, w_hh_b, b_b, w_fc, b_fc):
    in_maps, per_core_runs, r_pc = host_inputs(
        text, text_lengths, emb, w_ih_f, w_hh_f, b_f, w_ih_b, w_hh_b, b_b)
    nc = get_runner(r_pc, reps=1)
    pools = run_on_device(nc, in_maps)
    return host_finish(pools, per_core_runs, text, text_lengths, emb,
                       w_ih_f, b_f, w_ih_b, b_b, w_fc, b_fc)
